# Initial kernel scaffold
#
"""LSTM cell (B=4096, D=U=2048) on 8 trn2 NeuronCores.

Tensor-parallel over units: core i computes units [i*256,(i+1)*256) of every
gate. Per core:
    z^T[1024 units, 4096 batch] = Wx_shard^T @ x^T + Wh_shard^T @ h^T
accumulated in PSUM (bf16 matmuls, fp32 accumulate), gate activations fused
with the bias add on ScalarE (units on partitions -> bias is per-partition),
elementwise LSTM combine on VectorE, outputs stored transposed and
re-transposed on the host.
"""

import sys

sys.path.insert(0, "/opt/trn_rl_repo")

import ml_dtypes
import numpy as np

import concourse.bass as bass
import concourse.mybir as mybir
import concourse.tile as tile
from bass_rust import ScopedClock
from concourse.bass_utils import run_bass_kernel_spmd

B, D, U = 4096, 2048, 2048
N_CORES = 8
US = U // N_CORES          # units per core per gate (256)
UT = US // 128             # unit tiles of 128 per gate (2)
NB = 512                   # batch tile (free dim)
NT = B // NB               # batch tiles (8)
KX = D // 128              # k tiles for x gemm (16)
KH = U // 128              # k tiles for h gemm (16)
BF16 = mybir.dt.bfloat16
F32 = mybir.dt.float32
AF = mybir.ActivationFunctionType


def _patch_tile_drain():
    """This walrus build only accepts one sem-wait on CTRL-class (no-struct)
    instructions, but Tile's end-of-context drain accumulates one wait per
    semaphore lane. Spread the waits over extra sync nops, one wait each."""
    if getattr(tile.TileContext, "_drain_patched", False):
        return

    def _drain_and_barrier(self, tick_clock, wait_clock):
        drain_inst = self.nc.sync.drain()
        wait_clock.add_sem_waits(
            drain_inst.ins, ScopedClock({None: tick_clock.global_clock})
        )
        si = drain_inst.ins.sync_info
        waits = list(si.on_wait) if si is not None else []
        if len(waits) > 1:
            si.on_wait = waits[:1]
            for w in waits[1:]:
                nop = self.nc.sync.nop()
                nsi = nop.ins.sync_info
                if nsi is None:
                    nop.ins.sync_info = mybir.SyncInfo(on_wait=[w], on_update=[])
                else:
                    nsi.on_wait = [w]
        self.nc.all_engine_barrier()
        assert self.sems is not None
        popped = self.nc._tile_sem_poison_stack.pop()
        assert popped is self._sem_poison
        self.nc.clear_and_free_semaphores(list(self.sems.allocated().values()))
        self.nc.all_engine_barrier()

    tile.TileContext._drain_and_barrier = _drain_and_barrier
    tile.TileContext._drain_patched = True


def build_nc() -> bass.Bass:
    _patch_tile_drain()
    nc = bass.Bass()

    xT = nc.dram_tensor("xT", [D, B], BF16, kind="ExternalInput")
    hT = nc.dram_tensor("hT", [U, B], BF16, kind="ExternalInput")
    wx = nc.dram_tensor("wx", [D, 4 * US], BF16, kind="ExternalInput")
    wh = nc.dram_tensor("wh", [U, 4 * US], BF16, kind="ExternalInput")
    # bias, host-prepped to [128, 8]: column j = units [j*128,(j+1)*128) of
    # the concatenated [f,i,o,g] 1024-unit block (gate j//2, unit-tile j%2)
    bias = nc.dram_tensor("bias", [128, 4 * UT], F32, kind="ExternalInput")
    cT = nc.dram_tensor("cT", [US, B], F32, kind="ExternalInput")
    h_newT = nc.dram_tensor("h_newT", [US, B], F32, kind="ExternalOutput")
    c_newT = nc.dram_tensor("c_newT", [US, B], F32, kind="ExternalOutput")

    wx_r = wx.rearrange("(kt p) u -> p kt u", p=128)  # [128, KX, 1024]
    wh_r = wh.rearrange("(kt p) u -> p kt u", p=128)
    xT_r = xT.rearrange("(kt p) b -> p kt b", p=128)  # [128, KX, B]
    hT_r = hT.rearrange("(kt p) b -> p kt b", p=128)

    with tile.TileContext(nc) as tc:
        with (
            tc.tile_pool(name="wpool", bufs=1) as wpool,
            tc.tile_pool(name="singles", bufs=1) as singles,
            tc.tile_pool(name="acts", bufs=3) as apool,
            tc.tile_pool(name="ew", bufs=3) as epool,
            tc.tile_pool(name="psum", bufs=8, space="PSUM") as ppool,
        ):
            b_sb = singles.tile([128, 4 * UT], F32)
            nc.sync.dma_start(out=b_sb[:], in_=bias[:])

            # resident weights, one tile per k-tile for fine-grained deps
            wx_t = []
            wh_t = []
            for kt in range(KX):
                wt = wpool.tile([128, 4 * US], BF16, tag=f"wx{kt}")
                nc.sync.dma_start(out=wt[:], in_=wx_r[:, kt, :])
                wx_t.append(wt)
            for kt in range(KH):
                wt = wpool.tile([128, 4 * US], BF16, tag=f"wh{kt}")
                nc.sync.dma_start(out=wt[:], in_=wh_r[:, kt, :])
                wh_t.append(wt)

            for n in range(NT):
                nsl = bass.ts(n, NB)
                x_sb = apool.tile([128, KX, NB], BF16, tag="x_sb")
                nc.sync.dma_start(out=x_sb[:], in_=xT_r[:, :, nsl])
                h_sb = apool.tile([128, KH, NB], BF16, tag="h_sb")
                nc.sync.dma_start(out=h_sb[:], in_=hT_r[:, :, nsl])

                for ut in range(UT):
                    usl = slice(ut * 128, (ut + 1) * 128)
                    gates = []
                    for gi in range(4):
                        c0 = gi * US + ut * 128
                        ps = ppool.tile([128, NB], F32, tag="ps")
                        for kt in range(KX):
                            nc.tensor.matmul(
                                ps[:],
                                wx_t[kt][:, c0 : c0 + 128],
                                x_sb[:, kt, :],
                                start=(kt == 0),
                                stop=False,
                            )
                        for kt in range(KH):
                            nc.tensor.matmul(
                                ps[:],
                                wh_t[kt][:, c0 : c0 + 128],
                                h_sb[:, kt, :],
                                start=False,
                                stop=(kt == KH - 1),
                            )
                        g_sb = epool.tile([128, NB], F32, tag=f"gate{gi}")
                        nc.scalar.activation(
                            g_sb[:],
                            ps[:],
                            AF.Tanh if gi == 3 else AF.Sigmoid,
                            bias=b_sb[:, gi * UT + ut : gi * UT + ut + 1],
                        )
                        gates.append(g_sb)
                    f_t, i_t, o_t, g_t = gates

                    c_sb = epool.tile([128, NB], F32, tag="c_sb")
                    nc.sync.dma_start(out=c_sb[:], in_=cT[usl, nsl])

                    # c_new = f*c + i*g
                    nc.vector.tensor_mul(f_t[:], f_t[:], c_sb[:])
                    nc.vector.tensor_mul(i_t[:], i_t[:], g_t[:])
                    cn = epool.tile([128, NB], F32, tag="cn")
                    nc.vector.tensor_add(cn[:], f_t[:], i_t[:])
                    nc.sync.dma_start(out=c_newT[usl, nsl], in_=cn[:])

                    # h_new = o * tanh(c_new)
                    nc.scalar.activation(g_t[:], cn[:], AF.Tanh)
                    nc.vector.tensor_mul(o_t[:], o_t[:], g_t[:])
                    nc.sync.dma_start(out=h_newT[usl, nsl], in_=o_t[:])
    return nc


_NC_CACHE = None


def _get_nc():
    global _NC_CACHE
    if _NC_CACHE is None:
        _NC_CACHE = build_nc()
    return _NC_CACHE


def make_in_maps(x, h, c, Wxf, Wxi, Wxo, Wxg, bf, bi, bo, bg, Whf, Whi, Who, Whg):
    bf16 = ml_dtypes.bfloat16
    xT = np.ascontiguousarray(np.asarray(x, np.float32).T).astype(bf16)
    hT = np.ascontiguousarray(np.asarray(h, np.float32).T).astype(bf16)
    c = np.asarray(c, np.float32)
    Wx = np.stack([np.asarray(w, np.float32) for w in (Wxf, Wxi, Wxo, Wxg)])
    Wh = np.stack([np.asarray(w, np.float32) for w in (Whf, Whi, Who, Whg)])
    bias = np.stack([np.asarray(v, np.float32) for v in (bf, bi, bo, bg)])

    in_maps = []
    for i in range(N_CORES):
        s = slice(i * US, (i + 1) * US)
        wx_i = np.concatenate([Wx[g, :, s] for g in range(4)], axis=1).astype(bf16)
        wh_i = np.concatenate([Wh[g, :, s] for g in range(4)], axis=1).astype(bf16)
        b_i = np.concatenate([bias[g, s] for g in range(4)])  # [1024]
        b_i = np.ascontiguousarray(b_i.reshape(4 * UT, 128).T)  # [128, 8]
        cT_i = np.ascontiguousarray(c[:, s].T)  # [US, B]
        in_maps.append(
            {"xT": xT, "hT": hT, "wx": wx_i, "wh": wh_i, "bias": b_i, "cT": cT_i}
        )
    return in_maps


def run(in_maps, **kwargs):
    nc = _get_nc()
    return run_bass_kernel_spmd(nc, in_maps, list(range(N_CORES)), **kwargs)


def gather(results):
    h_new = np.empty((B, U), np.float32)
    c_new = np.empty((B, U), np.float32)
    for i in range(N_CORES):
        s = slice(i * US, (i + 1) * US)
        h_new[:, s] = results[i]["h_newT"].T
        c_new[:, s] = results[i]["c_newT"].T
    return h_new, c_new


def kernel(**inputs):
    res = run(make_in_maps(**inputs))
    return gather(res.results)


# revision 4
# speedup vs baseline: 1.0222x; 1.0222x over previous
"""LSTM cell (B=4096, D=U=2048) on 8 trn2 NeuronCores.

Tensor-parallel over units: core i computes units [i*256,(i+1)*256) of every
gate. Per core:
    z^T[1024 units, 4096 batch] = Wx_shard^T @ x^T + Wh_shard^T @ h^T
accumulated in PSUM (bf16 matmuls, fp32 accumulate), gate activations fused
with the bias add on ScalarE (units on partitions -> bias is per-partition),
elementwise LSTM combine on VectorE, outputs stored transposed and
re-transposed on the host.
"""

import sys

sys.path.insert(0, "/opt/trn_rl_repo")

import ml_dtypes
import numpy as np

import concourse.bass as bass
import concourse.mybir as mybir
import concourse.tile as tile
from concourse.bass_utils import run_bass_kernel_spmd

B, D, U = 4096, 2048, 2048
N_CORES = 8
US = U // N_CORES          # units per core per gate (256)
UT = US // 128             # unit tiles of 128 per gate (2)
NB = 512                   # batch tile (free dim)
NT = B // NB               # batch tiles (8)
KX = D // 128              # k tiles for x gemm (16)
KH = U // 128              # k tiles for h gemm (16)
BF16 = mybir.dt.bfloat16
F32 = mybir.dt.float32
AF = mybir.ActivationFunctionType


def _split_excess_waits(nc, maxw=1):
    """This walrus build rejects instructions carrying more than one sem-wait
    ("Too many sync wait commands"), but Tile freely attaches several. Hoist
    the extra waits onto same-engine nops inserted right before the
    instruction — engine streams are in-order, so blocking semantics are
    identical."""
    cnt = 0
    for fn in nc.m.functions:
        for bb in fn.blocks:
            new_insts = []
            for inst in bb.instructions:
                si = inst.sync_info
                waits = list(si.on_wait) if si is not None else []
                if len(waits) > maxw:
                    for i in range(0, len(waits) - maxw, maxw):
                        nop = mybir.InstNoOp(name=f"syncsplit-{cnt}")
                        cnt += 1
                        nop.engine = inst.engine
                        nop.sync_info = mybir.SyncInfo(
                            on_wait=waits[i : i + maxw], on_update=[]
                        )
                        new_insts.append(nop)
                    si.on_wait = waits[len(waits) - maxw :]
                new_insts.append(inst)
            if len(new_insts) != len(bb.instructions):
                bb.instructions = new_insts
    return cnt


def build_nc() -> bass.Bass:
    nc = bass.Bass()

    xT = nc.dram_tensor("xT", [D, B], BF16, kind="ExternalInput")
    hT = nc.dram_tensor("hT", [U, B], BF16, kind="ExternalInput")
    wx = nc.dram_tensor("wx", [D, 4 * US], BF16, kind="ExternalInput")
    wh = nc.dram_tensor("wh", [U, 4 * US], BF16, kind="ExternalInput")
    # bias, host-prepped to [128, 8]: column j = units [j*128,(j+1)*128) of
    # the concatenated [f,i,o,g] 1024-unit block (gate j//2, unit-tile j%2)
    bias = nc.dram_tensor("bias", [128, 4 * UT], F32, kind="ExternalInput")
    cT = nc.dram_tensor("cT", [US, B], F32, kind="ExternalInput")
    h_newT = nc.dram_tensor("h_newT", [US, B], F32, kind="ExternalOutput")
    c_newT = nc.dram_tensor("c_newT", [US, B], F32, kind="ExternalOutput")

    wx_r = wx.rearrange("(kt p) u -> p kt u", p=128)  # [128, KX, 1024]
    wh_r = wh.rearrange("(kt p) u -> p kt u", p=128)
    xT_r = xT.rearrange("(kt p) b -> p kt b", p=128)  # [128, KX, B]
    hT_r = hT.rearrange("(kt p) b -> p kt b", p=128)

    with tile.TileContext(nc) as tc:
        with (
            tc.tile_pool(name="wpool", bufs=1) as wpool,
            tc.tile_pool(name="singles", bufs=1) as singles,
            tc.tile_pool(name="acts", bufs=2) as apool,
            tc.tile_pool(name="ew", bufs=3) as epool,
            tc.tile_pool(name="psum", bufs=8, space="PSUM") as ppool,
        ):
            b_sb = singles.tile([128, 4 * UT], F32)
            nc.sync.dma_start(out=b_sb[:], in_=bias[:])

            # resident weights, one tile per k-tile for fine-grained deps
            wx_t = []
            wh_t = []
            for kt in range(KX):
                wt = wpool.tile([128, 4 * US], BF16, tag=f"wx{kt}")
                nc.sync.dma_start(out=wt[:], in_=wx_r[:, kt, :])
                wx_t.append(wt)
            for kt in range(KH):
                wt = wpool.tile([128, 4 * US], BF16, tag=f"wh{kt}")
                nc.sync.dma_start(out=wt[:], in_=wh_r[:, kt, :])
                wh_t.append(wt)

            for n in range(NT):
                nsl = bass.ts(n, NB)
                x_sb = apool.tile([128, KX, NB], BF16, tag="x_sb")
                nc.sync.dma_start(out=x_sb[:], in_=xT_r[:, :, nsl])
                h_sb = apool.tile([128, KH, NB], BF16, tag="h_sb")
                nc.sync.dma_start(out=h_sb[:], in_=hT_r[:, :, nsl])

                for ut in range(UT):
                    usl = slice(ut * 128, (ut + 1) * 128)
                    gates = []
                    for gi in range(4):
                        c0 = gi * US + ut * 128
                        ps = ppool.tile([128, NB], F32, tag="ps")
                        for kt in range(KX):
                            nc.tensor.matmul(
                                ps[:],
                                wx_t[kt][:, c0 : c0 + 128],
                                x_sb[:, kt, :],
                                start=(kt == 0),
                                stop=False,
                            )
                        for kt in range(KH):
                            nc.tensor.matmul(
                                ps[:],
                                wh_t[kt][:, c0 : c0 + 128],
                                h_sb[:, kt, :],
                                start=False,
                                stop=(kt == KH - 1),
                            )
                        g_sb = epool.tile([128, NB], F32, tag=f"gate{gi}")
                        nc.scalar.activation(
                            g_sb[:],
                            ps[:],
                            AF.Tanh if gi == 3 else AF.Sigmoid,
                            bias=b_sb[:, gi * UT + ut : gi * UT + ut + 1],
                        )
                        gates.append(g_sb)
                    f_t, i_t, o_t, g_t = gates

                    c_sb = epool.tile([128, NB], F32, tag="c_sb")
                    nc.sync.dma_start(out=c_sb[:], in_=cT[usl, nsl])

                    # c_new = f*c + i*g
                    nc.vector.tensor_mul(f_t[:], f_t[:], c_sb[:])
                    nc.vector.tensor_mul(i_t[:], i_t[:], g_t[:])
                    cn = epool.tile([128, NB], F32, tag="cn")
                    nc.vector.tensor_add(cn[:], f_t[:], i_t[:])
                    nc.sync.dma_start(out=c_newT[usl, nsl], in_=cn[:])

                    # h_new = o * tanh(c_new)
                    nc.scalar.activation(g_t[:], cn[:], AF.Tanh)
                    nc.vector.tensor_mul(o_t[:], o_t[:], g_t[:])
                    nc.sync.dma_start(out=h_newT[usl, nsl], in_=o_t[:])
    _split_excess_waits(nc)
    return nc


_NC_CACHE = None


def _get_nc():
    global _NC_CACHE
    if _NC_CACHE is None:
        _NC_CACHE = build_nc()
    return _NC_CACHE


def make_in_maps(x, h, c, Wxf, Wxi, Wxo, Wxg, bf, bi, bo, bg, Whf, Whi, Who, Whg):
    bf16 = ml_dtypes.bfloat16
    xT = np.ascontiguousarray(np.asarray(x, np.float32).T).astype(bf16)
    hT = np.ascontiguousarray(np.asarray(h, np.float32).T).astype(bf16)
    c = np.asarray(c, np.float32)
    Wx = np.stack([np.asarray(w, np.float32) for w in (Wxf, Wxi, Wxo, Wxg)])
    Wh = np.stack([np.asarray(w, np.float32) for w in (Whf, Whi, Who, Whg)])
    bias = np.stack([np.asarray(v, np.float32) for v in (bf, bi, bo, bg)])

    in_maps = []
    for i in range(N_CORES):
        s = slice(i * US, (i + 1) * US)
        wx_i = np.concatenate([Wx[g, :, s] for g in range(4)], axis=1).astype(bf16)
        wh_i = np.concatenate([Wh[g, :, s] for g in range(4)], axis=1).astype(bf16)
        b_i = np.concatenate([bias[g, s] for g in range(4)])  # [1024]
        b_i = np.ascontiguousarray(b_i.reshape(4 * UT, 128).T)  # [128, 8]
        cT_i = np.ascontiguousarray(c[:, s].T)  # [US, B]
        in_maps.append(
            {"xT": xT, "hT": hT, "wx": wx_i, "wh": wh_i, "bias": b_i, "cT": cT_i}
        )
    return in_maps


def run(in_maps, **kwargs):
    nc = _get_nc()
    return run_bass_kernel_spmd(nc, in_maps, list(range(N_CORES)), **kwargs)


def gather(results):
    h_new = np.empty((B, U), np.float32)
    c_new = np.empty((B, U), np.float32)
    for i in range(N_CORES):
        s = slice(i * US, (i + 1) * US)
        h_new[:, s] = results[i]["h_newT"].T
        c_new[:, s] = results[i]["c_newT"].T
    return h_new, c_new


def kernel(**inputs):
    res = run(make_in_maps(**inputs))
    return gather(res.results)
